# revision 1
# baseline (speedup 1.0000x reference)
"""Trainium2 Bass kernel for nn_LinearLLM: out[b,t,v] = sum_{s>=t,w} x[b,s,w]*W[s,w,t,v] + bias.

Strategy: one uniform SPMD program on 8 cores. t-axis sharded cyclically
(core c owns t in {c, c+8, c+16, ...}) so the causal-mask work per core is
identical -> same instruction stream, different data. The contraction
(s,w) is chunked into 257 K-chunks of 128; chunk k only touches the
core's t-columns with t <= 2k+1 (prefix of width N_k = 6*((2k+1)//8+1)),
halving both DMA and FLOPs. Weights are packed on host into a per-core
bf16 slab grouped 8 chunks per DMA (contiguous per partition) and the
whole contraction accumulates in a single PSUM bank.

The embedding lookup x = embedding[src] is computed ON DEVICE from a
tiny one-hot tensor (V=6 token values): slot j covers chunks {2j, 2j+1};
a K=32 matmul of a block-diagonal embedding lhsT against one-hot columns
produces 256 batch-columns of x^T per MM, accumulated into PSUM then
copied (cast to bf16) into the XT operand buffer. This removes the
8.4MB/core XT DMA entirely.
"""
import numpy as np
import ml_dtypes

from concourse import bacc, tile
from concourse.bass_utils import run_bass_kernel_spmd
import concourse.mybir as mybir

B, L1, EMB, V, NCORES = 128, 513, 64, 6, 8
S_PAD = L1 + 1                 # pad s to even
NCHUNK = S_PAD // 2            # 257 K-chunks of 128 (2 s-values x 64 w)
NPAIR = (NCHUNK + 3) // 4      # 65 one-hot pairs (4 chunks each)
OH_COLS = NPAIR * 512          # 65 col-blocks of 512
CNT = 65                       # padded t-count per core (core 0 has 65, rest 64)
NCOLS = CNT * V                # 390 output columns per core
GRP = 16                        # K-chunks per weight DMA group

MM_DT = mybir.dt.bfloat16
NP_DT = ml_dtypes.bfloat16


def _width(k):
    """Unmasked column-prefix width for K-chunk k (uniform over cores)."""
    return min(6 * ((2 * k + 1) // 8 + 1), NCOLS)


def _groups():
    gs = []
    for k0 in range(0, NCHUNK, GRP):
        k1 = min(k0 + GRP, NCHUNK)
        gs.append((k0, k1, _width(k1 - 1)))
    return gs


GROUPS = _groups()
SLAB_TOTAL = sum(128 * _width(k) for k in range(NCHUNK))

_CACHE = {}


def _build():
    if "nc" in _CACHE:
        return _CACHE["nc"]
    nc = bacc.Bacc("TRN2", target_bir_lowering=False, debug=False,
                   num_devices=NCORES)
    ebr_dram = nc.declare_dram_parameter("ebr", [128, 128], MM_DT,
                                         isOutput=False)
    oh_dram = nc.declare_dram_parameter("oh", [128, OH_COLS], MM_DT,
                                        isOutput=False)
    slab_dram = nc.declare_dram_parameter("slab", [SLAB_TOTAL], MM_DT,
                                          isOutput=False)
    out_dram = nc.declare_dram_parameter("out", [128, NCOLS],
                                         mybir.dt.float32, isOutput=True)

    with tile.TileContext(nc) as tc:
        with (
            tc.tile_pool(name="xtp", bufs=16) as xtp,
            tc.tile_pool(name="wp", bufs=8) as wp,
            tc.tile_pool(name="op", bufs=1) as op,
            tc.tile_pool(name="psum", bufs=1, space="PSUM") as psp,
            tc.tile_pool(name="pxp", bufs=6, space="PSUM") as pxp,
        ):
            ebr = op.tile([128, 128], MM_DT)
            oh = op.tile([128, OH_COLS], MM_DT)
            nc.sync.dma_start(ebr[:], ebr_dram[:])
            nc.sync.dma_start(oh[:], oh_dram[:])

            # xt[p, k, b] = x[b, 2k + p//64, p%64], bf16, built on device:
            # pair p covers chunks 4p..4p+3 with one K=64 one-hot matmul
            # (N=512, alternating 64-row bases so LDWEIGHTS overlaps),
            # interleaved with the masked contraction so the weight-slab
            # DMA streams continuously from t=0.
            xts = []
            for p in range(NPAIR):
                xt_pair = xtp.tile([128, 4, 128], MM_DT, tag="xt")
                xts.append(xt_pair)
            zero = op.tile([128, NCOLS], MM_DT)
            nc.gpsimd.memset(zero[:], 0.0)
            NB = 2                       # psum accumulator banks
            pss = []
            for a in range(NB):
                ps_a = psp.tile([128, NCOLS], mybir.dt.float32, tag=f"ps{a}")
                pss.append(ps_a)
                nc.tensor.matmul(ps_a[:], zero[:, :128], zero[:],
                                 start=True, stop=False)

            def build_pair(p):
                px = pxp.tile([128, 512], mybir.dt.float32, tag="px")
                rb = 64 * (p % 2)
                nc.tensor.matmul(px[:], ebr[rb:rb + 64, :],
                                 oh[rb:rb + 64, p * 512:(p + 1) * 512],
                                 start=True, stop=True)
                nck = min(4, NCHUNK - 4 * p)
                if p % 2 == 0:
                    nc.vector.tensor_copy(xts[p][:, :nck, :],
                                          px[:, :nck * 128])
                else:
                    nc.scalar.copy(xts[p][:, :nck, :], px[:, :nck * 128])

            built = 0
            off = 0
            for k0, k1, w in GROUPS:
                while built * 4 < k1:
                    build_pair(built)
                    built += 1
                wsum = sum(_width(k) for k in range(k0, k1))
                wt = wp.tile([128, wsum], MM_DT, tag="w")
                src_ap = slab_dram[off:off + 128 * wsum].rearrange(
                    "(p n) -> p n", p=128)
                nc.sync.dma_start(wt[:], src_ap)
                off += 128 * wsum
                ok = 0
                for k in range(k0, k1):
                    wk = _width(k)
                    nc.tensor.matmul(pss[k % NB][:, :wk],
                                     xts[k // 4][:, k % 4, :],
                                     wt[:, ok:ok + wk],
                                     start=False,
                                     stop=(k >= NCHUNK - NB))
                    ok += wk

            o = op.tile([128, NCOLS], mybir.dt.float32)
            nc.vector.tensor_copy(o[:], pss[0][:])
            for a in range(1, NB):
                nc.vector.tensor_add(o[:], o[:], pss[a][:])
            nc.sync.dma_start(out_dram[:], o[:])

    nc.compile()
    _CACHE["nc"] = nc
    return nc


def _prep_shared(src, embedding):
    """ebr (block-diag embedding lhsT) and oh (one-hot rhs), both bf16."""
    src = np.asarray(src)
    E = np.asarray(embedding, dtype=np.float32)

    # ebr[16h + smod*6 + c, smod*64 + w] = E[c, w] for every 16-row sub-slot
    ebr = np.zeros((128, 128), np.float32)
    for h in range(8):
        for smod in range(2):
            r0 = 16 * h + smod * 6
            ebr[r0:r0 + V, smod * EMB:(smod + 1) * EMB] = E

    # oh[64*(p%2) + 16*(k%4) + smod*6 + c, p*512 + (k%4)*128 + b] = 1
    #   where p = k//4, c = src[b, 2k+smod]  (s=513 pad -> c=6 -> zero ebr row)
    idxp = np.full((B, S_PAD), V, np.int64)
    idxp[:, :L1] = src
    kk = np.arange(NCHUNK)
    pp = kk // 4
    bb = np.arange(B)
    oh = np.zeros((128, OH_COLS), np.float32)
    for smod in range(2):
        cvals = idxp[:, 2 * kk + smod]               # (B, NCHUNK)
        rows = (64 * (pp % 2) + 16 * (kk % 4) + 6 * smod)[None, :] + cvals
        cols = (pp * 512 + (kk % 4) * 128)[None, :] + bb[:, None]
        oh[rows.ravel(), cols.ravel()] = 1.0
    return ebr.astype(NP_DT), oh.astype(NP_DT)


def _prep_inputs(src, embedding, weight):
    weight = np.asarray(weight, dtype=np.float32)
    ebr, oh = _prep_shared(src, embedding)

    s_idx = np.arange(S_PAD)
    in_maps = []
    for c in range(NCORES):
        cnt = len(range(c, L1, 8))
        wc = np.zeros((S_PAD, EMB, CNT, V), np.float32)
        wc[:L1, :, :cnt, :] = weight[:, :, c::8, :]
        tj = np.arange(CNT) * 8 + c
        mask = (s_idx[:, None] >= tj[None, :])
        wc *= mask[:, None, :, None]
        wc2 = wc.reshape(S_PAD, EMB, CNT * V)
        parts = []
        for k0, k1, w in GROUPS:
            cols = [wc2[2 * k:2 * k + 2, :, :_width(k)].reshape(128, -1)
                    for k in range(k0, k1)]
            blk = np.concatenate(cols, axis=1)
            parts.append(np.ascontiguousarray(blk).reshape(-1).astype(NP_DT))
        slab = np.concatenate(parts)
        in_maps.append({"ebr": ebr, "oh": oh, "slab": slab})
    return in_maps


def _unshard(results, bias):
    full = np.zeros((B, L1, V), np.float32)
    for c in range(NCORES):
        cnt = len(range(c, L1, 8))
        oc = results[c]["out"].reshape(B, CNT, V)
        full[:, c::8, :] = oc[:, :cnt, :]
    full += np.asarray(bias, dtype=np.float32)[None]
    return np.ascontiguousarray(full.transpose(0, 2, 1))


def kernel(src, embedding, weight, bias):
    nc = _build()
    in_maps = _prep_inputs(src, embedding, weight)
    res = run_bass_kernel_spmd(nc, in_maps, list(range(NCORES)))
    return _unshard(res.results, bias)



# revision 3
# speedup vs baseline: 2.1185x; 2.1185x over previous
"""Trainium2 Bass kernel for nn_LinearLLM: out[b,t,v] = sum_{s>=t,w} x[b,s,w]*W[s,w,t,v] + bias.

Strategy: shard the CONTRACTION axis s across the 8 cores (cyclic over
128-row K-chunks = 2 s-values x 64 w), each core computing partial sums
for ALL 3078 = 513*6 output (t,v) columns; the 8 bf16 partials are summed
on host. This cuts LDWEIGHTS to 33 loads/core (vs 257 for t-sharding)
and gives every matmul a wide moving operand.

Weights are quantized to fp8 e3m4 (x2048, power of two) with a greedy
error-feedback rounding that near-cancels the quantization error inside
the 6-dim subspace spanned by the (also e3m4) embedding rows — measured
end-to-end rel err ~3e-3 vs 1.8e-2 for round-to-nearest. fp8 halves the
weight DMA stream (6.5MB/core), the binding resource.

Per core: 32 regular K-chunks in DESCENDING s order (position i has
uniform padded width 96*(32-i) so all cores run one SPMD program) + a
1/8 column slice of the final s=512 chunk. PSUM banks 0-5 accumulate the
3072 main columns; as s descends, high-t columns stop receiving
contributions, so banks drain (cast bf16 + DMA out) progressively while
compute continues.
"""
import numpy as np
import ml_dtypes

from concourse import bacc, tile
from concourse.bass_utils import run_bass_kernel_spmd
import concourse.mybir as mybir

B, L1, EMB, V, NCORES = 128, 513, 64, 6, 8
NPOS = 32                      # regular K-chunk positions per core
SCALE = 2048.0                 # weight scale 2^11 (exact rescale on host)
NCOLS = 3072                   # main out cols (t < 512), 6 banks x 512
XCOLS = 385                    # per-core col slice of the s=512 chunk
OUTC = NCOLS + XCOLS
W_DT = mybir.dt.float8e3
NP_W = ml_dtypes.float8_e3m4

WIDTHS = [96 * (NPOS - i) for i in range(NPOS)]          # 3072 ... 96
XT_COLS = (NPOS + 1) * 128                               # 4224


def _groups():
    gs, cur, acc = [], [], 0
    for i in range(NPOS):
        cur.append(i)
        acc += WIDTHS[i]
        if acc >= 4096:
            gs.append(cur)
            cur, acc = [], 0
    if cur:
        gs.append(cur)
    return gs


GROUPS = _groups()
SLAB_MAIN = 128 * sum(WIDTHS)
SLAB_TOTAL = SLAB_MAIN + 64 * XCOLS
# last position whose width still covers psum bank j (drain point)
I_STOP = [max(i for i in range(NPOS) if WIDTHS[i] > 512 * j) for j in range(6)]

_CACHE = {}


def _build():
    if "nc" in _CACHE:
        return _CACHE["nc"]
    nc = bacc.Bacc("TRN2", target_bir_lowering=False, debug=False,
                   num_devices=NCORES)
    xt_dram = nc.declare_dram_parameter("xt", [128, XT_COLS], W_DT,
                                        isOutput=False)
    slab_dram = nc.declare_dram_parameter("slab", [SLAB_TOTAL], W_DT,
                                          isOutput=False)
    out_dram = nc.declare_dram_parameter("out", [128, OUTC],
                                         mybir.dt.bfloat16, isOutput=True)

    def slab_ap(off, n):
        return slab_dram[off:off + 128 * n].rearrange("(p n) -> p n", p=128)

    with tile.TileContext(nc) as tc:
        with (
            tc.tile_pool(name="io", bufs=1) as iop,
            tc.tile_pool(name="ps", bufs=1, space="PSUM") as psp,
        ):
            NA = 8                     # positions in the first xt piece
            xtA = iop.tile([128, NA * 128], W_DT, tag="xtA")
            xtB = iop.tile([128, (NPOS + 1 - NA) * 128], W_DT, tag="xtB")
            wgs = [iop.tile([128, sum(WIDTHS[i] for i in g)], W_DT,
                            tag=f"g{gi}", name=f"wg{gi}")
                   for gi, g in enumerate(GROUPS)]
            w32 = iop.tile([64, XCOLS], W_DT, tag="w32")
            obufs = [iop.tile([128, 512], mybir.dt.bfloat16, tag=f"o{j}",
                              name=f"ob{j}") for j in range(6)]
            ox = iop.tile([128, XCOLS], mybir.dt.bfloat16, tag="ox")
            pss = [psp.tile([128, 512], mybir.dt.float32, tag=f"ps{j}",
                            name=f"ps{j}") for j in range(6)]
            psx = psp.tile([128, XCOLS], mybir.dt.float32, tag="psx")

            # --- queue all input DMAs (ring processes them in order) ---
            nc.sync.dma_start(xtA[:], xt_dram[:, :NA * 128])
            off = 0
            goffs = []
            for gi, g in enumerate(GROUPS):
                goffs.append(off)
                wsum = sum(WIDTHS[i] for i in g)
                nc.sync.dma_start(wgs[gi][:], slab_ap(off, wsum))
                off += 128 * wsum
                if gi == 0:
                    nc.sync.dma_start(xtB[:], xt_dram[:, NA * 128:])
                if gi == 1:
                    nc.sync.dma_start(
                        w32[:],
                        slab_dram[SLAB_MAIN:SLAB_MAIN + 64 * XCOLS]
                        .rearrange("(p n) -> p n", p=64))

            def lhsT(i):
                if i < NA:
                    return xtA[:, 128 * i:128 * (i + 1)]
                return xtB[:, 128 * (i - NA):128 * (i - NA + 1)]

            drain_seq = [0, 1]         # scalar, vector alternation
            def drain(j, src_ps, obuf, cols, dst0):
                if drain_seq[0] % 2 == 0:
                    nc.scalar.copy(obuf[:, :cols], src_ps[:, :cols])
                else:
                    nc.vector.tensor_copy(obuf[:, :cols], src_ps[:, :cols])
                drain_seq[0] += 1
                nc.scalar.dma_start(out_dram[:, dst0:dst0 + cols],
                                    obuf[:, :cols])

            # --- contraction: positions descending in s ---
            for gi, g in enumerate(GROUPS):
                off_in_g = 0
                for i in g:
                    w_i = WIDTHS[i]
                    for j in range((w_i + 511) // 512):
                        c0, c1 = 512 * j, min(512 * (j + 1), w_i)
                        nc.tensor.matmul(
                            pss[j][:, :c1 - c0],
                            lhsT(i),
                            wgs[gi][:, off_in_g + c0:off_in_g + c1],
                            start=(i == 0),
                            stop=(i == I_STOP[j]),
                        )
                    off_in_g += w_i
                    if i == 8:
                        # s=512 chunk (K=64), own bank, single matmul
                        nc.tensor.matmul(psx[:], xtB[0:64, 3072:3200],
                                         w32[0:64, :], start=True, stop=True)
                        drain(-1, psx, ox, XCOLS, NCOLS)
                    for j in range(6):
                        if I_STOP[j] == i:
                            drain(j, pss[j], obufs[j], 512, 512 * j)

    nc.compile()
    _CACHE["nc"] = nc
    return nc


def _quantize_weights(emb, W):
    """Greedy error-feedback e3m4 quantization of SCALE*W.

    Returns (xq8 (6,64) e3m4, Wq8 (513,513,6,64) e3m4 scaled, masked t<=s).
    Rounding of each 64-element w-row chooses floor/ceil per element to
    cancel the running residual r = A(q-w) + b0 where A = dequantized
    e3m4 embedding and b0 compensates the embedding's own quant error.
    """
    emb = np.asarray(emb, np.float32)
    W = np.asarray(W, np.float32)
    xq8 = emb.astype(NP_W)
    xq = xq8.astype(np.float32)                     # (6,64)
    ex = xq - emb

    Ws = W * SCALE                                  # (513,64,513,6) fp32
    B0 = np.tensordot(ex, Ws, axes=([1], [1]))      # (6,513,513,6)

    Wr = np.ascontiguousarray(Ws.transpose(0, 2, 3, 1)).reshape(-1, EMB)
    del Ws
    s_idx = np.repeat(np.arange(L1), L1 * V)
    t_idx = np.tile(np.repeat(np.arange(L1), V), L1)
    valid = t_idx <= s_idx
    Wv = np.ascontiguousarray(Wr[valid])            # (Nv, 64)
    r = np.ascontiguousarray(
        B0.transpose(1, 2, 3, 0).reshape(-1, V)[valid])
    del B0

    allb = np.arange(256, dtype=np.uint8)
    vals = allb.view(NP_W).astype(np.float32)
    grid = np.unique(vals[np.isfinite(vals)])
    lo_i = np.searchsorted(grid, Wv, side="right") - 1
    lo = grid[np.clip(lo_i, 0, len(grid) - 1)]
    hi = grid[np.clip(lo_i + 1, 0, len(grid) - 1)]
    del lo_i
    Q = np.empty_like(Wv)

    A = xq.T.copy()                                 # (64, 6)
    order = np.argsort(-np.linalg.norm(A, axis=1))
    for j in order:
        aj = A[j]
        n2 = float(aj @ aj)
        g = r @ aj
        dlo = lo[:, j] - Wv[:, j]
        dhi = hi[:, j] - Wv[:, j]
        pick_hi = 2 * g * dhi + dhi * dhi * n2 < 2 * g * dlo + dlo * dlo * n2
        d = np.where(pick_hi, dhi, dlo)
        Q[:, j] = np.where(pick_hi, hi[:, j], lo[:, j])
        r += d[:, None] * aj
    for j in order:                                 # one refinement sweep
        aj = A[j]
        n2 = float(aj @ aj)
        g = r @ aj
        cur = Q[:, j]
        other = np.where(cur == lo[:, j], hi[:, j], lo[:, j])
        dd = other - cur
        flip = 2 * g * dd + dd * dd * n2 < 0
        Q[:, j] = np.where(flip, other, cur)
        r += np.where(flip, dd, 0.0)[:, None] * aj

    Wq = np.zeros_like(Wr)
    Wq[valid] = Q
    Wq8 = Wq.reshape(L1, L1, V, EMB).astype(NP_W)   # (s,t,v,w)
    return xq8, Wq8


def _prep_inputs(src, embedding, weight):
    src = np.asarray(src)
    xq8, Wq8 = _quantize_weights(embedding, weight)

    xfull = xq8[src]                                # (B, 513, 64) e3m4
    row512 = np.ascontiguousarray(
        Wq8[512].transpose(2, 0, 1)).reshape(EMB, L1 * V)   # (64, 3078)

    in_maps = []
    for c in range(NCORES):
        ks = [8 * (NPOS - 1 - i) + c for i in range(NPOS)]
        s_arr = np.array([[2 * k, 2 * k + 1] for k in ks])   # (32,2)
        sel = xfull[:, s_arr, :]                     # (B,32,2,64)
        xt = np.zeros((128, NPOS + 1, 128), NP_W)
        xt[:, :NPOS, :] = sel.transpose(2, 3, 1, 0).reshape(128, NPOS, B)
        xt[:EMB, NPOS, :] = xfull[:, 512, :].T
        xt2 = np.ascontiguousarray(xt.reshape(128, XT_COLS))

        parts = []
        for g in GROUPS:
            blks = []
            for i in g:
                k = ks[i]
                t_hi = WIDTHS[i] // V
                arr = Wq8[2 * k:2 * k + 2, :t_hi, :, :]     # (2,t_hi,6,64)
                blks.append(arr.transpose(0, 3, 1, 2).reshape(128, WIDTHS[i]))
            parts.append(np.ascontiguousarray(
                np.concatenate(blks, axis=1)).reshape(-1))
        w32 = np.zeros((64, XCOLS), NP_W)
        c0 = XCOLS * c
        c1 = min(c0 + XCOLS, L1 * V)
        w32[:, :c1 - c0] = row512[:, c0:c1]
        parts.append(w32.reshape(-1))
        slab = np.concatenate(parts)
        assert slab.shape[0] == SLAB_TOTAL
        in_maps.append({"xt": xt2, "slab": slab})
    return in_maps


def _unshard(results, bias):
    full = np.zeros((B, L1 * V), np.float32)
    for c in range(NCORES):
        o = results[c]["out"].astype(np.float32)
        full[:, :NCOLS] += o[:, :NCOLS]
        c0 = XCOLS * c
        c1 = min(c0 + XCOLS, L1 * V)
        full[:, c0:c1] += o[:, NCOLS:NCOLS + (c1 - c0)]
    full *= 1.0 / SCALE
    full = full.reshape(B, L1, V) + np.asarray(bias, np.float32)[None]
    return np.ascontiguousarray(full.transpose(0, 2, 1))


def kernel(src, embedding, weight, bias):
    nc = _build()
    in_maps = _prep_inputs(src, embedding, weight)
    res = run_bass_kernel_spmd(nc, in_maps, list(range(NCORES)))
    return _unshard(res.results, bias)
